# revision 62
# baseline (speedup 1.0000x reference)
"""DiffusionAttentionPairBias kernel for Trainium2 (8 NeuronCores, SPMD).

Problem (B=1, N=1024, D_A=768, D_S=384, D_Z=128, H=16, DH=48):
  q_in = sigmoid(LN(s) @ gw_ad + gb_ad) * LN(a) + LN(s) @ bw_ad
  q,k,v,g = projections of q_in;  bias = (LN(z)*zn_g + zn_b) @ zp_w
  attn = softmax(q k^T / sqrt(DH) + bias);  out = sigmoid(g) * (attn v)
  y = sigmoid(s @ sg_w + sg_b) * (out @ ow)

Sharding: pure data-parallel on the query axis. Core c owns query rows
[128c, 128c+128): it receives the full a/s (to build k/v for all keys),
its own 128-row slices a_q/s_q/z_q, and computes its 128 output rows.
No collectives; host concatenates.

Structure (per core):
 - z (32MB bf16) is loaded HBM->SBUF directly through the xbar TRANSPOSE
   DMA, landing as [ch, k, q] tiles (one 512KB transpose per 16 kp).
   The z pipeline is emitted first: it is the long pole and its DMAs
   should start at t=0; the prologue fills engine gaps.
 - LN over z's ch axis is folded into the projection. Per 4-kp chunk c,
   two matmuls accumulate into PSUM partitions 32c..32c+31:
     A (stationary [128,32] = [zn_g*zp_w | 1 | 0...]) @ zT   -> P0, S1
     B (stationary [128,32] = [0x17 | 1 | 0...])      @ zT^2 -> S2
   The squares stream zT^2 is computed per 4-kp chunk on DVE/ACT/GPSIMD.
 - The [128,512] PSUM tile (4 col-groups x 32) is evacuated 128-wide to
   bf16 and xbar-transposed back to [q, kpos, 32] layout (braw): slot
   0-15 = P0[h], 16 = S1, 17 = S2, where kpos permutes keys within each
   16-block (kappa<->c swap). kT/v_sb use the same key permutation, to
   which softmax/AV are invariant. The LN correction
     bias = (P0 - S1*colsum(W)/128) / sqrt(S2/128 - (S1/128)^2 + eps)
   is applied in place per 256-key quarter as soon as its groups land
   (zn_b cancels in softmax).
 - Flash-style attention: each head's half runs as soon as that braw
   half is corrected; partial AV is parked in SBUF (avh) so the two
   attn tile slots recycle per half. exp on ACT with accum_out giving
   the softmax denominator for free; denominator and sigmoid-gate fold
   into the AV epilogue.
 - Weights are pre-cast/pre-fused on the host (bf16, qk-scale folded
   into qw/qb, kw padded to 64 cols/head so K-proj matmuls fill all
   128 PSUM rows).
"""

import math
import os

import ml_dtypes
import numpy as np

import concourse.bass as bass
import concourse.bacc as bacc
import concourse.mybir as mybir
import concourse.tile as tile
from concourse.masks import make_identity

F32 = mybir.dt.float32
BF16 = mybir.dt.bfloat16
AF = mybir.ActivationFunctionType
ALU = mybir.AluOpType
AX = mybir.AxisListType

N = 1024
DA = 768
DS = 384
DZ = 128
H = 16
DH = 48
HD = 768
QP = 128          # query rows per core
NCORES = 8
SCALE = 1.0 / math.sqrt(DH)
EPS = 1e-5
KJ = 16           # kp rows per z macro-tile
NJ = N // KJ      # 64 z macro-tiles
GT = 4            # z macro-tiles per bias back-transpose group
NG = NJ // GT     # 16 transpose groups


def _bcast(ap, dim, n):
    """Return a copy of `ap` whose `dim`-th AP dim is replaced by [0, n]."""
    dims = [list(d) for d in ap.ap]
    dims[dim] = [0, n]
    return bass.AP(tensor=ap.tensor, offset=ap.offset, ap=dims)


def _with_dims(ap, dims):
    """Return a copy of `ap` (keeping partition dim) with given free dims
    [[stride, n], ...] (strides in elements)."""
    return bass.AP(
        tensor=ap.tensor,
        offset=ap.offset,
        ap=[list(ap.ap[0])] + [list(d) for d in dims],
    )


def _permk(ap, nblocks):
    """Key-permuted view of a contiguous [128, 16*nblocks] slice: position
    16b + 4*kappa + c reads element 16b + 4c + kappa (kappa<->c swap within
    each 16-block, matching the z bias back-transpose output order)."""
    return _with_dims(ap, [[16, nblocks], [1, 4], [4, 4]])


def build_program():
    nc = bacc.Bacc("TRN2", target_bir_lowering=False, debug=False)

    def din(name, shape, dt=F32):
        return nc.dram_tensor(name, shape, dt, kind="ExternalInput")

    a_d = din("a", [N, DA])
    s_d = din("s", [N, DS])
    aq_d = din("a_q", [QP, DA])
    sq_d = din("s_q", [QP, DS])
    z_d = din("z_q", [QP, N, DZ], BF16)
    gwad_d = din("adaln_gw", [DS, DA], BF16)
    bwad_d = din("adaln_bw", [DS, DA], BF16)
    gbad_d = din("adaln_gb", [1, DA], BF16)
    qw_d = din("qw", [DA, HD], BF16)          # pre-scaled by 1/sqrt(DH)
    qb_d = din("qb", [1, HD], BF16)           # pre-scaled
    kwp_d = din("kw", [DA, H * 64], BF16)     # 64-padded per head
    vw_d = din("vw", [DA, HD], BF16)
    gw_d = din("gw", [DA, HD], BF16)
    ow_d = din("ow", [HD, DA], BF16)
    sgw_d = din("sg_w", [DS, DA], BF16)
    sgb_d = din("sg_b", [1, DA], BF16)
    waugA_d = din("waugA", [DZ, 32], BF16)    # [zn_g*zp_w | 1 | 0]
    waugB_d = din("waugB", [DZ, 32], BF16)    # [0 x17 | 1 | 0]
    csd_d = din("csd", [1, H], BF16)          # colsum(zn_g*zp_w)/DZ
    out_d = nc.dram_tensor("out", [QP, DA], F32, kind="ExternalOutput")

    with tile.TileContext(nc) as tc:
        with (
            tc.tile_pool(name="const", bufs=1) as cp,
            tc.tile_pool(name="persist", bufs=1) as pp,
            tc.tile_pool(name="wpool", bufs=2) as wp,
            tc.tile_pool(name="act", bufs=2) as ap_,
            tc.tile_pool(name="ztr", bufs=3) as ztp,
            tc.tile_pool(name="zsqp", bufs=2) as zqp,
            tc.tile_pool(name="stage", bufs=1) as stp,
            tc.tile_pool(name="attnp", bufs=2) as atp,
            tc.tile_pool(name="ps_z", bufs=2, space="PSUM") as ps_z,
            tc.tile_pool(name="ps_s", bufs=2, space="PSUM") as ps_s,
            tc.tile_pool(name="ps_m", bufs=2, space="PSUM") as ps_m,
        ):
            # ---------------- constants / small prep ----------------
            ones_r = cp.tile([1, 128], BF16)
            nc.vector.memset(ones_r, 1.0)
            epsA = cp.tile([128, 1], F32)
            nc.vector.memset(epsA, EPS)
            ident = cp.tile([128, 128], BF16)
            make_identity(nc, ident)

            waugA = cp.tile([DZ, 32], BF16)
            nc.sync.dma_start(out=waugA, in_=waugA_d[:])
            waugB = cp.tile([DZ, 32], BF16)
            nc.sync.dma_start(out=waugB, in_=waugB_d[:])
            csd_r = cp.tile([1, H], BF16)
            nc.sync.dma_start(out=csd_r, in_=csd_d[:])
            # csD: colsum(W)/DZ broadcast to all 128 partitions (rank-1 mm)
            ps_cs = ps_m.tile([128, H], F32, tag="misc")
            nc.tensor.matmul(ps_cs, ones_r[:], csd_r[:], start=True, stop=True)
            csD = cp.tile([128, H], BF16)
            nc.vector.tensor_copy(csD, ps_cs[:])

            gbad_r = cp.tile([1, DA], BF16)
            nc.sync.dma_start(out=gbad_r, in_=gbad_d[:])
            qb_r = cp.tile([1, HD], BF16)
            nc.sync.dma_start(out=qb_r, in_=qb_d[:])
            sgb_r = cp.tile([1, DA], BF16)
            nc.sync.dma_start(out=sgb_r, in_=sgb_d[:])

            # ---------------- persistent activations ----------------
            # s_lnT/a_ln5 die once adaln is done; share slots with v_sb/kT
            s_lnT = pp.tile([128, 3, N], BF16, tag="vsb")
            q_inT = pp.tile([128, 6, N], BF16, tag="big")  # q_in^T (slot
            # later reused by ow_s: ow load waits for q_in's last reader)
            qi_qT = pp.tile([128, 6, QP], BF16)       # q_in^T, this core's rows
            kT = pp.tile([128, 8, N], BF16, tag="kT")  # K^T head-pairs 0/64
            qT = pp.tile([128, 8, QP], BF16)          # (Q*s+qb*s)^T head-pairs
            v_sb = pp.tile([128, 8, HD], BF16, tag="vsb")
            sig_g = pp.tile([128, HD], BF16)
            sig_o = pp.tile([128, DA], BF16)
            out_nat = pp.tile([128, HD], BF16)
            braw = pp.tile([128, N, 32], BF16)        # [q, kpos, (P0|S1|S2|.)]
            avh = pp.tile([128, H, DH], BF16)         # half-0 partial AV
            den16 = pp.tile([128, H, 4], F32)

            # ---------------- prologue (fills z-loop gaps) ----------------
            def ln_tile(src_ap, cols, out_bf):
                """LayerNorm rows of [128, cols] -> bf16 tile (no affine)."""
                xt = ap_.tile([128, cols], F32, tag="lnin")
                nc.sync.dma_start(out=xt, in_=src_ap)
                st6 = ap_.tile([128, 2, 6], F32, tag="lnst")
                half = cols // 2
                nc.vector.bn_stats(out=st6[:, 0, :], in_=xt[:, 0:half])
                nc.vector.bn_stats(out=st6[:, 1, :], in_=xt[:, half:cols])
                mv = ap_.tile([128, 2], F32, tag="lnmv")
                nc.vector.bn_aggr(out=mv, in_=st6[:, :, :])
                sd = ap_.tile([128, 1], F32, tag="lnsd")
                nc.scalar.activation(sd, mv[:, 1:2], AF.Sqrt, bias=epsA[:])
                rs = ap_.tile([128, 1], F32, tag="lnrs")
                nc.vector.reciprocal(rs, sd[:])
                nc.vector.tensor_scalar(
                    out=out_bf,
                    in0=xt[:],
                    scalar1=mv[:, 0:1],
                    scalar2=rs[:],
                    op0=ALU.subtract,
                    op1=ALU.mult,
                )

            a_ln5 = pp.tile([128, 5, DA], BF16, tag="kT")
            for t in range(8):
                s_ln = ap_.tile([128, DS], BF16, tag="sln")
                ln_tile(s_d[t * 128 : (t + 1) * 128, :], DS, s_ln[:])
                nc.sync.dma_start(
                    out=s_lnT[:, :, t * 128 : (t + 1) * 128],
                    in_=s_ln[:],
                    transpose=True,
                )
            sq_ln = pp.tile([128, DS], BF16)
            ln_tile(sq_d[:], DS, sq_ln[:])
            sq_lnT = pp.tile([128, 3, QP], BF16)
            nc.sync.dma_start(out=sq_lnT[:, :, :], in_=sq_ln[:], transpose=True)

            # ---- adaln -> q_in (all positions) -> q_inT ----
            gbw_s = wp.tile([128, 6, DA], BF16, tag="w9")
            nc.sync.dma_start(
                out=gbw_s[:, 0:3, :],
                in_=gwad_d[:].rearrange("(t p) n -> p t n", p=128),
            )
            nc.sync.dma_start(
                out=gbw_s[:, 3:6, :],
                in_=bwad_d[:].rearrange("(t p) n -> p t n", p=128),
            )

            chunks = [(0, 512), (512, 256)]

            def adaln_qin(lnT_ap, a_tile, a_i, out_T, outT_col0):
                """q_in rows for 128 positions; lnT_ap(kt) -> [128,128] lhsT.
                Chunked PSUM tiles (1 bank each) so 2 chunks pipeline."""
                q_in = ap_.tile([128, DA], BF16, tag="qtmp")
                for c0, cn in chunks:
                    psG = ps_m.tile([128, 512], F32, tag="misc")
                    for kt in range(3):
                        nc.tensor.matmul(
                            psG[:, 0:cn],
                            lnT_ap(kt),
                            gbw_s[:, kt, c0 : c0 + cn],
                            start=(kt == 0),
                            stop=False,
                        )
                    nc.tensor.matmul(
                        psG[:, 0:cn],
                        ones_r[:],
                        gbad_r[:, c0 : c0 + cn],
                        start=False,
                        stop=True,
                    )
                    sgG = ap_.tile([128, 512], BF16, tag="sgG")
                    nc.scalar.activation(sgG[:, 0:cn], psG[:, 0:cn], AF.Sigmoid)
                    psB = ps_m.tile([128, 512], F32, tag="misc")
                    for kt in range(3):
                        nc.tensor.matmul(
                            psB[:, 0:cn],
                            lnT_ap(kt),
                            gbw_s[:, 3 + kt, c0 : c0 + cn],
                            start=(kt == 0),
                            stop=(kt == 2),
                        )
                    nc.vector.tensor_mul(
                        q_in[:, c0 : c0 + cn], sgG[:, 0:cn],
                        a_tile[:, a_i, c0 : c0 + cn],
                    )
                    nc.vector.tensor_add(
                        q_in[:, c0 : c0 + cn], q_in[:, c0 : c0 + cn],
                        psB[:, 0:cn],
                    )
                nc.sync.dma_start(
                    out=out_T[:, :, outT_col0 : outT_col0 + 128],
                    in_=q_in[:],
                    transpose=True,
                )

            # Two waves: LN(a) (Sqrt set) for a wave, then its adaln
            # (Sigmoid set) — bounds a_ln SBUF at 5 tiles, ~4 table loads.
            for t in range(4):
                ln_tile(a_d[t * 128 : (t + 1) * 128, :], DA, a_ln5[:, t, :])
            for t in range(4):
                adaln_qin(
                    lambda kt, t=t: s_lnT[:, kt, t * 128 : (t + 1) * 128],
                    a_ln5, t, q_inT, t * 128,
                )
            for t in range(4, 8):
                ln_tile(a_d[t * 128 : (t + 1) * 128, :], DA, a_ln5[:, t - 4, :])
            ln_tile(aq_d[:], DA, a_ln5[:, 4, :])
            for t in range(4, 8):
                adaln_qin(
                    lambda kt, t=t: s_lnT[:, kt, t * 128 : (t + 1) * 128],
                    a_ln5, t - 4, q_inT, t * 128,
                )
            # q-row version (recomputed from a_q/s_q so the program is SPMD)
            adaln_qin(lambda kt: sq_lnT[:, kt, :], a_ln5, 4, qi_qT, 0)

            # ---- K^T head-pairs (kw host-padded to 64 cols/head so each MM
            # fills all 128 output rows: pair p = rows 0-47 & 64-111) ----
            for pw in range(2):
                kw_s = wp.tile([128, 6, 512], BF16, tag="w9")
                nc.sync.dma_start(
                    out=kw_s,
                    in_=kwp_d[:, 512 * pw : 512 * (pw + 1)].rearrange(
                        "(t p) n -> p t n", p=128
                    ),
                )
                for p in range(4 * pw, 4 * pw + 4):
                    pl = p - 4 * pw
                    for half in range(2):
                        c0 = half * 512
                        psK = ps_m.tile([128, 512], F32, tag="misc")
                        for kt in range(6):
                            nc.tensor.matmul(
                                psK[:, :],
                                kw_s[:, kt, 128 * pl : 128 * pl + 128],
                                q_inT[:, kt, c0 : c0 + 512],
                                start=(kt == 0),
                                stop=(kt == 5),
                            )
                        nc.vector.tensor_copy(
                            _permk(kT[:, p, c0 : c0 + 512], 32), psK[:, :]
                        )

            # ---- V natural (key-permuted rows) ----
            vw_s = wp.tile([128, 6, HD], BF16, tag="w9")
            nc.sync.dma_start(
                out=vw_s, in_=vw_d[:].rearrange("(t p) n -> p t n", p=128)
            )
            for t in range(8):
                for c0, cn in chunks:
                    psV = ps_m.tile([128, 512], F32, tag="misc")
                    for kt in range(6):
                        nc.tensor.matmul(
                            psV[:, 0:cn],
                            q_inT[:, kt, t * 128 : (t + 1) * 128],
                            vw_s[:, kt, c0 : c0 + cn],
                            start=(kt == 0),
                            stop=(kt == 5),
                        )
                    nc.vector.tensor_copy(v_sb[:, t, c0 : c0 + cn], psV[:, 0:cn])

            # ---- Q^T head-pairs (scale pre-folded into qw/qb) ----
            qw_s = wp.tile([128, 6, HD], BF16, tag="w9")
            nc.sync.dma_start(
                out=qw_s, in_=qw_d[:].rearrange("(t p) n -> p t n", p=128)
            )
            for p in range(8):
                psQ = ps_m.tile([128, QP], F32, tag="misc")
                for sub in range(2):
                    h = 2 * p + sub
                    off = 64 * sub
                    for kt in range(6):
                        nc.tensor.matmul(
                            psQ[off : off + 48, :],
                            qw_s[:, kt, 48 * h : 48 * h + 48],
                            qi_qT[:, kt, :],
                            start=(kt == 0),
                            stop=False,
                        )
                    nc.tensor.matmul(
                        psQ[off : off + 48, :],
                        qb_r[:, 48 * h : 48 * h + 48],
                        ones_r[:],
                        start=False,
                        stop=True,
                    )
                for sub in range(2):
                    off = 64 * sub
                    nc.vector.tensor_copy(
                        qT[off : off + 48, p, :], psQ[off : off + 48, :]
                    )

            # ---- G gate ----
            gw_s = wp.tile([128, 6, HD], BF16, tag="w9")
            nc.sync.dma_start(
                out=gw_s, in_=gw_d[:].rearrange("(t p) n -> p t n", p=128)
            )
            for c0, cn in chunks:
                psg = ps_m.tile([128, 512], F32, tag="misc")
                for kt in range(6):
                    nc.tensor.matmul(
                        psg[:, 0:cn],
                        qi_qT[:, kt, :],
                        gw_s[:, kt, c0 : c0 + cn],
                        start=(kt == 0),
                        stop=(kt == 5),
                    )
                nc.scalar.activation(sig_g[:, c0 : c0 + cn], psg[:, 0:cn], AF.Sigmoid)

            # ---- output gate from raw s_q ----
            sgw_s = wp.tile([128, 3, DA], BF16, tag="w9")
            nc.sync.dma_start(
                out=sgw_s, in_=sgw_d[:].rearrange("(t p) n -> p t n", p=128)
            )
            sq_f = ap_.tile([128, DS], F32, tag="lnin")
            nc.sync.dma_start(out=sq_f, in_=sq_d[:])
            sq_bf = ap_.tile([128, DS], BF16, tag="sqbf")
            nc.vector.tensor_copy(sq_bf, sq_f[:])
            sqT = pp.tile([128, 3, QP], BF16)
            nc.sync.dma_start(out=sqT[:, :, :], in_=sq_bf[:], transpose=True)
            for c0, cn in chunks:
                pso = ps_m.tile([128, 512], F32, tag="misc")
                for kt in range(3):
                    nc.tensor.matmul(
                        pso[:, 0:cn],
                        sqT[:, kt, :],
                        sgw_s[:, kt, c0 : c0 + cn],
                        start=(kt == 0),
                        stop=False,
                    )
                nc.tensor.matmul(
                    pso[:, 0:cn], ones_r[:], sgb_r[:, c0 : c0 + cn],
                    start=False, stop=True,
                )
                nc.scalar.activation(sig_o[:, c0 : c0 + cn], pso[:, 0:cn], AF.Sigmoid)

            # ---------------- z pipeline: pair bias ----------------
            def corrections(qtr):
                """LN correction, in place, for 256-key quarter `qtr`."""
                QK4 = 256
                k0 = qtr * QK4
                S1 = braw[:, k0 : k0 + QK4, 16:17]
                S2 = braw[:, k0 : k0 + QK4, 17:18]
                P0 = braw[:, k0 : k0 + QK4, 0:16]
                x1 = ap_.tile([128, QK4, 1], F32, tag="x1", bufs=1)
                nc.vector.tensor_mul(x1, S1, S1)
                x2 = ap_.tile([128, QK4, 1], F32, tag="x2", bufs=1)
                nc.vector.scalar_tensor_tensor(
                    out=x2, in0=x1[:], scalar=-1.0 / DZ, in1=S2,
                    op0=ALU.mult, op1=ALU.add,
                )
                nc.vector.tensor_scalar_max(out=x2[:], in0=x2[:], scalar1=0.0)
                sdv = ap_.tile([128, QK4, 1], F32, tag="zsd", bufs=1)
                nc.scalar.activation(
                    sdv, x2[:], AF.Sqrt, scale=1.0 / DZ, bias=epsA[:]
                )
                rstd = ap_.tile([128, QK4, 1], F32, tag="zrs", bufs=1)
                nc.vector.reciprocal(rstd, sdv[:])
                t1 = ap_.tile([128, QK4, 16], BF16, tag="t1", bufs=1)
                nc.vector.tensor_mul(
                    t1, _bcast(S1, 2, 16),
                    _bcast(csD[:].rearrange("p h -> p () h"), 1, QK4),
                )
                nc.vector.tensor_sub(P0, P0, t1[:])
                nc.vector.tensor_mul(P0, P0, _bcast(rstd[:], 2, 16))

            # squares engine pattern per 8 j's: 6x DVE, 1x ACT, 1x GPSIMD
            sq_eng = [0, 1, 0, 1, 0, 1, 0, 1]
            for g in range(NG):
                sg = stp.tile([128, GT, 512], BF16, tag="sg")
                for j2 in range(GT):
                    j = g * GT + j2
                    k0 = j * KJ
                    zTt = ztp.tile([128, KJ, 128], BF16, tag="zT")
                    nc.sync.dma_start(
                        out=zTt,
                        in_=z_d[:, k0 : k0 + KJ, :].rearrange("q a b -> q (a b)"),
                        transpose=True,
                    )
                    zsq = zqp.tile([128, KJ, 128], BF16, tag="zsq")
                    e = sq_eng[j % 8]
                    P = ps_z.tile([128, 512], F32, tag="p0", bufs=3)
                    for c in range(4):
                        zc = zTt[:, 4 * c : 4 * c + 4, :]
                        rz = zc.rearrange("p a b -> p (a b)")
                        zq_c = zsq[:, 4 * c : 4 * c + 4, :]
                        # per-chunk squares: B's matmul only waits on its own
                        # 512-col chunk, overlapping squares with A's matmuls
                        if e == 0:
                            nc.vector.tensor_mul(zq_c, zc, zc)
                        elif e == 1:
                            nc.scalar.activation(zq_c, zc, AF.Square)

                        rq = zq_c.rearrange("p a b -> p (a b)")
                        nc.tensor.matmul(
                            P[32 * c : 32 * c + 32, :], waugA[:], rz,
                            start=True, stop=False, tile_position=(0, 32 * c),
                        )
                        nc.tensor.matmul(
                            P[32 * c : 32 * c + 32, :], waugB[:], rq,
                            start=False, stop=True, tile_position=(0, 32 * c),
                        )
                    if j % 2 == 0:
                        nc.vector.tensor_copy(sg[:, j2, :], P[:])
                    else:
                        nc.scalar.activation(sg[:, j2, :], P[:], AF.Copy)
                # back-transpose into braw (kpos = kappa<->c swapped key
                # order); kT columns and attn columns are permuted to match
                # via DVE/ACT output APs (matmul operands stay 1-free-dim,
                # which walrus requires).
                nc.sync.dma_start(
                    out=_with_dims(
                        braw[:, 64 * g : 64 * (g + 1), :], [[128, 16], [1, 128]]
                    ),
                    in_=sg[:].rearrange("p a b -> p (a b)"),
                    transpose=True,
                )
                # corrections as soon as each quarter's groups are done
                if g in (3, 7, 11, 15):
                    corrections(g // 4)

            # ---------------- attention (flash-style halves) ----------------
            def attn_half(h, half):
                po = 64 * (h % 2)
                pr = h // 2
                c0 = half * 512
                attn = atp.tile([128, 512], BF16, tag=f"attn{h % 6}", bufs=1)
                attnT = atp.tile([128, 4, 128], BF16, tag=f"attnT{h % 6}", bufs=1)
                sc = ps_s.tile([128, 512], F32, tag="sc")
                nc.tensor.matmul(
                    sc,
                    qT[po : po + 48, pr, :],
                    kT[po : po + 48, pr, c0 : c0 + 512],
                    start=True,
                    stop=False,
                )
                nc.tensor.matmul(
                    sc,
                    ident[:],
                    braw[:, c0 : c0 + 512, h : h + 1].rearrange("p a b -> p (a b)"),
                    start=False,
                    stop=True,
                )
                # |logits| small for this problem: exp w/o max-subtract
                nc.scalar.activation(
                    _permk(attn[:], 32), sc[:], AF.Exp,
                    accum_out=den16[:, h, half : half + 1],
                )
                nc.sync.dma_start(out=attnT[:], in_=attn[:], transpose=True)
                psA = ps_z.tile([128, DH], F32, tag="psA", bufs=1)
                for kt in range(4):
                    nc.tensor.matmul(
                        psA,
                        attnT[:, kt, :],
                        v_sb[:, kt + 4 * half, DH * h : DH * h + DH],
                        start=(kt == 0),
                        stop=(kt == 3),
                    )
                if half == 0:
                    nc.vector.tensor_copy(avh[:, h, :], psA[:])
                else:
                    nc.vector.tensor_add(
                        den16[:, h, 2:3], den16[:, h, 0:1], den16[:, h, 1:2]
                    )
                    rden = den16[:, h, 3:4]
                    nc.vector.reciprocal(rden, den16[:, h, 2:3])
                    pav = ap_.tile([128, DH], F32, tag="pav")
                    nc.vector.tensor_add(pav, psA[:], avh[:, h, :])
                    nc.vector.scalar_tensor_tensor(
                        out=out_nat[:, DH * h : DH * h + DH],
                        in0=pav[:],
                        scalar=rden,
                        in1=sig_g[:, DH * h : DH * h + DH],
                        op0=ALU.mult,
                        op1=ALU.mult,
                    )

            for h in range(H):
                attn_half(h, 0)
            for h in range(H):
                attn_half(h, 1)

            # ---------------- output projection ----------------
            outT = pp.tile([128, 6, QP], BF16)
            nc.sync.dma_start(out=outT[:, :, :], in_=out_nat[:], transpose=True)
            ow_s = pp.tile([128, 6, DA], BF16, tag="big")
            nc.sync.dma_start(
                out=ow_s, in_=ow_d[:].rearrange("(t p) n -> p t n", p=128)
            )
            fin = pp.tile([128, DA], F32)
            for c0, cn in chunks:
                psF = ps_m.tile([128, 512], F32, tag="misc")
                for kt in range(6):
                    nc.tensor.matmul(
                        psF[:, 0:cn],
                        outT[:, kt, :],
                        ow_s[:, kt, c0 : c0 + cn],
                        start=(kt == 0),
                        stop=(kt == 5),
                    )
                nc.vector.tensor_mul(
                    fin[:, c0 : c0 + cn], psF[:, 0:cn], sig_o[:, c0 : c0 + cn]
                )
            nc.sync.dma_start(out=out_d[:], in_=fin[:])

    nc.compile()
    return nc


_CACHE = {}


def _get_program():
    if "nc" not in _CACHE:
        _CACHE["nc"] = build_program()
    return _CACHE["nc"]


def _pad64(w):
    """Pad [DA, H*48] head-blocks to 64 cols/head -> [DA, H*64]."""
    out = np.zeros((w.shape[0], H * 64), np.float32)
    for h in range(H):
        out[:, 64 * h : 64 * h + DH] = w[:, DH * h : DH * h + DH]
    return out


def make_in_maps(inputs):
    """Shard full inputs into 8 per-core input maps (host-side prep)."""
    f32 = lambda k: np.ascontiguousarray(np.asarray(inputs[k], dtype=np.float32))
    bf = lambda x: np.ascontiguousarray(np.asarray(x, dtype=np.float32)).astype(
        ml_dtypes.bfloat16
    )
    a = f32("a")[0]
    s = f32("s")[0]
    z = np.asarray(inputs["z"], dtype=np.float32)[0].astype(ml_dtypes.bfloat16)

    zn_g = f32("zn_g")
    zp_w = f32("zp_w")
    W = zn_g[:, None] * zp_w                      # [DZ, H]
    waugA = np.zeros((DZ, 32), np.float32)
    waugA[:, :H] = W
    waugA[:, 16] = 1.0
    waugB = np.zeros((DZ, 32), np.float32)
    waugB[:, 17] = 1.0
    csd = (W.sum(axis=0) / DZ)[None, :]           # [1, H]

    shared = {
        "a": a,
        "s": s,
        "adaln_gw": bf(inputs["adaln_gw"]),
        "adaln_bw": bf(inputs["adaln_bw"]),
        "adaln_gb": bf(np.asarray(inputs["adaln_gb"]).reshape(1, DA)),
        "qw": bf(f32("qw") * SCALE),
        "qb": bf((f32("qb") * SCALE).reshape(1, HD)),
        "kw": bf(_pad64(f32("kw"))),
        "vw": bf(inputs["vw"]),
        "gw": bf(inputs["gw"]),
        "ow": bf(inputs["ow"]),
        "sg_w": bf(inputs["sg_w"]),
        "sg_b": bf(np.asarray(inputs["sg_b"]).reshape(1, DA)),
        "waugA": bf(waugA),
        "waugB": bf(waugB),
        "csd": bf(csd),
    }
    in_maps = []
    for c in range(NCORES):
        sl = slice(c * QP, (c + 1) * QP)
        m = dict(shared)
        m["a_q"] = np.ascontiguousarray(a[sl])
        m["s_q"] = np.ascontiguousarray(s[sl])
        m["z_q"] = np.ascontiguousarray(z[sl])
        in_maps.append(m)
    return in_maps


def kernel(**inputs) -> np.ndarray:
    from concourse.bass_utils import run_bass_kernel_spmd

    nc = _get_program()
    in_maps = make_in_maps(inputs)
    trace = bool(int(os.environ.get("KERNEL_TRACE", "0")))
    try:
        res = run_bass_kernel_spmd(
            nc, in_maps, core_ids=list(range(NCORES)), trace=trace
        )
    except ModuleNotFoundError:
        res = run_bass_kernel_spmd(
            nc, in_maps, core_ids=list(range(NCORES)), trace=False
        )
    _CACHE["last_results"] = res
    out = np.concatenate([res.results[c]["out"] for c in range(NCORES)], axis=0)
    return out[None].astype(np.float32)
